# revision 1
# baseline (speedup 1.0000x reference)
"""Trainium2 Bass kernel for nn_DagLinkExtractor (sparse_attention).

Math (per batch b, per row i):
  Q = F @ (Wq/16) + bq/16 ; K = F @ Wk + bk            (fp32r matmuls)
  s_h[i,j] = Q_h[i] . K_h[j]                            (= scores/sqrt(HD))
  gates: u_h = exp(gl_h - max_h gl), gden = sum_h u_h
  masked s: s + (-1e9) for (j<=i) | !valid[j]           (pen rank-1 MM + tri TT add)
  p_h = exp(s_masked), S_h = sum_j p_h                  (ACT Exp + accum_out)
  w_h = u_h / (gden * (S_h + 1e-30))
  out[i,j] = ln(sum_h p_h[i,j] * w_h + 1e-38)           (then min-masks -> exact -1e9)

Sharding: data-parallel over B, one batch per NeuronCore (8 cores), no
collectives.  Host prep: transpose F, scale Wq, round matmul operands to
fp32r (E8M11), build mask vectors.  Host post: fill the never-computed
block-lower-left region with -1e9.
"""
import numpy as np

import concourse.bass as bass
import concourse.mybir as mybir
import concourse.tile as tile
from concourse import bacc
from concourse.bass_utils import run_bass_kernel_spmd

f32 = mybir.dt.float32
f32r = mybir.dt.float32r

B, N, HID, NH = 8, 1024, 1024, 4
HD = HID // NH          # 256
NC = HID // 128         # 8 chunks of the hidden/contraction dim
NI = N // 128           # 8 row chunks
NEG = -1000000000.0
BIG = 3.0e9             # "valid" marker in min-masks; BIG - 1e9 > any ln value


def round_f32r(x: np.ndarray) -> np.ndarray:
    """Round fp32 -> fp32r (E8M11): RNE on the low 12 mantissa bits."""
    u = np.ascontiguousarray(x, dtype=np.float32).view(np.uint32).astype(np.uint64)
    r = (u + 0x7FF + ((u >> 12) & 1)) & 0xFFFFF000
    return r.astype(np.uint32).view(np.float32)


def _pin_act_tables():
    """Make natural_log_exp_and_others the only set offering Exp/Ln/Identity
    so bacc emits a single ACT table load instead of ping-ponging between the
    exp-only and ln-only sets every row chunk (~2.7us per switch)."""
    from concourse.hw_specs import get_activation_tables
    aft = mybir.ActivationFunctionType
    tables = get_activation_tables("gen3")  # functools.cache -> shared dict
    keep = "natural_log_exp_and_others"
    if keep in tables:
        for name, funcs in tables.items():
            if name != keep:
                funcs.discard(aft.Exp)
                funcs.discard(aft.Ln)
                funcs.discard(aft.Identity)


def build_nc(variant="full", reps=1):
    _pin_act_tables()
    nc = bacc.Bacc("TRN2", target_bir_lowering=False, debug=False)

    ft_d = nc.dram_tensor("ft", [HID, N], f32r, kind="ExternalInput").ap()
    wq_d = nc.dram_tensor("wq", [HID, HID], f32r, kind="ExternalInput").ap()
    wk_d = nc.dram_tensor("wk", [HID, HID], f32r, kind="ExternalInput").ap()
    wg_d = nc.dram_tensor("wg", [HID, NH], f32r, kind="ExternalInput").ap()
    # packed f32 consts: [tri | tribig | bq | bk]  (128, 272)
    cp_d = nc.dram_tensor("cp", [128, 2 * 128 + 2 * NC], f32,
                          kind="ExternalInput").ap()
    # packed f32r row consts: [pen | ones | bg]  (1, N + 128 + NH)
    rp_d = nc.dram_tensor("rp", [1, N + 128 + NH], f32r,
                          kind="ExternalInput").ap()
    # packed f32r mats: [identity | tri-additive]  (128, 256)
    im_d = nc.dram_tensor("im", [128, 256], f32r, kind="ExternalInput").ap()
    # mv: BIG where valid, -1e9 where !valid — final min-mask (broadcast)
    mv_d = nc.dram_tensor("mv", [1, N], f32, kind="ExternalInput").ap()
    out_d = nc.dram_tensor("out", [N, N], f32, kind="ExternalOutput").ap()

    with tile.TileContext(nc) as tc:
        with tc.tile_pool(name="keep", bufs=1) as keep:

            # ---- persistent SBUF tensors ----
            qt = keep.tile([128, NC, N], f32r, tag="qt")   # Q^T (d, i), d-chunked
            kt = keep.tile([128, NC, N], f32r, tag="kt")   # K^T (d, j)
            mvb = keep.tile([128, N], f32, tag="mvb")      # broadcast min-mask
            cp_t = keep.tile([128, 2 * 128 + 2 * NC], f32, tag="cp")
            rp_t = keep.tile([1, N + 128 + NH], f32r, tag="rp")
            im_t = keep.tile([128, 256], f32r, tag="im")
            ident_t = im_t[:, 0:128]
            trir_t = im_t[:, 128:256]
            tribig_t = cp_t[:, 128:256]
            bq_t = cp_t[:, 256:256 + NC]
            bk_t = cp_t[:, 256 + NC:256 + 2 * NC]
            pen_t = rp_t[:, 0:N]
            ones_t = rp_t[:, N:N + 128]
            bg_t = rp_t[:, N + 128:N + 128 + NH]
            u_t = keep.tile([128, NI, NH], f32, tag="u")   # gate numerators
            gd_t = keep.tile([128, NI], f32, tag="gd")     # gate denominators
            eps_t = keep.tile([128, 1], f32, tag="eps")
            nc.vector.memset(eps_t[:, :], 1e-38)

            def load_consts():
                nc.scalar.dma_start(out=cp_t[:, :], in_=cp_d)
                nc.scalar.dma_start(out=rp_t[:, :], in_=rp_d)
                nc.scalar.dma_start(out=im_t[:, :], in_=im_d)
                nc.scalar.dma_start(out=mvb[:, :], in_=bass.AP(
                    tensor=mv_d.tensor, offset=mv_d.offset, ap=[[0, 128], [1, N]]))

            for _rep in range(reps):
                _emit_body(nc, tc, keep, variant,
                           ft_d, wq_d, wk_d, wg_d, out_d,
                           qt, kt, mvb, ident_t, trir_t, tribig_t, ones_t,
                           pen_t, bg_t, bq_t, bk_t, u_t, gd_t, eps_t,
                           load_consts if _rep == 0 else None)

    nc.compile()
    return nc


def _emit_body(nc, tc, keep, variant, ft_d, wq_d, wk_d, wg_d, out_d,
               qt, kt, mvb, ident_t, trir_t, tribig_t, ones_t, pen_t,
               bg_t, bq_t, bk_t, u_t, gd_t, eps_t, load_consts=None):
    with tc.tile_pool(name="wts", bufs=1) as wts, \
         tc.tile_pool(name="psum", bufs=8, space="PSUM") as psp:
        ft = wts.tile([128, NC, N], f32r, tag="ft", name="ft")
        wq = wts.tile([128, NC, HID], f32r, tag="wq", name="wq")
        wk = wts.tile([128, NC, HID], f32r, tag="wk", name="wk")
        wg = wts.tile([128, NC, NH], f32r, tag="wg", name="wg")
        # chunked loads, Q-critical tensors first, so projections can start
        # as soon as the first contraction chunks land
        ft_r = ft_d.rearrange("(a p) n -> p a n", p=128)
        wq_r = wq_d.rearrange("(a p) n -> p a n", p=128)
        wk_r = wk_d.rearrange("(a p) n -> p a n", p=128)
        for c in range(NC):
            nc.sync.dma_start(out=ft[:, c, :], in_=ft_r[:, c, :])
            nc.scalar.dma_start(out=wq[:, c, :], in_=wq_r[:, c, :])
        if load_consts is not None:
            load_consts()
        for c in range(NC):
            nc.sync.dma_start(out=wk[:, c, :], in_=wk_r[:, c, :])
        nc.scalar.dma_start(
            out=wg[:, :, :], in_=wg_d.rearrange("(a p) n -> p a n", p=128))

        # ---- projections: qt[d, i] = sum_c W[c, d] * ft[c, i] (+bias) ----
        # c-outer over all 8 d-chunks at once (one psum bank each) so the PE
        # streams useful matmuls as soon as each input chunk's DMA lands.
        for (w_t, b_t, o_t, on_dve) in ((wq, bq_t, qt, False),
                                        (wk, bk_t, kt, False)):
            for ih in range(2):
                pss = []
                for dc in range(NC):
                    ps = psp.tile([128, 512], f32, tag="proj", name="proj_ps")
                    pss.append(ps)
                for c in range(NC):
                    for dc in range(NC):
                        nc.tensor.matmul(
                            pss[dc][:, :],
                            w_t[:, c, dc * 128:(dc + 1) * 128],
                            ft[:, c, ih * 512:(ih + 1) * 512],
                            start=(c == 0), stop=(c == NC - 1))
                for dc in range(NC):
                    dst = o_t[:, dc, ih * 512:(ih + 1) * 512]
                    if on_dve:
                        nc.vector.tensor_scalar(
                            out=dst, in0=pss[dc][:, :],
                            scalar1=b_t[:, dc:dc + 1], scalar2=None,
                            op0=mybir.AluOpType.add)
                    else:
                        nc.scalar.activation(
                            dst, pss[dc][:, :],
                            mybir.ActivationFunctionType.Identity,
                            bias=b_t[:, dc:dc + 1], scale=1.0)

        # ---- gates ----
        for ic in range(NI):
            gps = psp.tile([128, 512], f32, tag="proj", name="gate_ps")[:, 0:NH]
            for c in range(NC):
                nc.tensor.matmul(
                    gps[:, :], ft[:, c, ic * 128:(ic + 1) * 128],
                    wg[:, c, :], start=(c == 0), stop=False)
            nc.tensor.matmul(gps[:, :], ones_t[:, :], bg_t[:, :],
                             start=False, stop=True)
            gnm = keep.tile([128, 1], f32, tag="gnm", name="gnm", bufs=4)
            nc.vector.reduce_max(gnm[:, :], gps[:, :],
                                 axis=mybir.AxisListType.X, negate=True)
            nc.scalar.activation(
                u_t[:, ic, :], gps[:, :],
                mybir.ActivationFunctionType.Exp,
                bias=gnm[:, 0:1], scale=1.0,
                accum_out=gd_t[:, ic:ic + 1])

    if variant == "noscores":
        for ic in range(NI):
            nc.sync.dma_start(out=out_d[ic * 128:(ic + 1) * 128, :],
                              in_=qt[:, ic, :].bitcast(f32))
        return

    # ---- per-row-chunk scores + masked softmax + head mixture ----
    with tc.tile_pool(name="wrk", bufs=3) as wrk, \
         tc.tile_pool(name="spsum", bufs=8, space="PSUM") as sps:
        for ic in (0, 4, 1, 5, 2, 6, 3, 7):
            jt0 = ic // 4            # first live 512-tile of j
            j0 = ic * 128            # first live column
            W = N - j0               # live width
            # per-(jt, head) chains: MMs -> pen -> tri -> exp.  jt-outer keeps
            # at most 4 psum banks per group so chunks pipeline across ic.
            p_ts = [wrk.tile([128, W], f32, tag=f"p{h}", name=f"p{h}")
                    for h in range(NH)]
            sa = wrk.tile([128, 2, NH], f32, tag="sa", name="sa")
            for k, jt in enumerate(range(jt0, 2)):
                lo = j0 - jt * 512 if jt == jt0 else 0
                dst0 = jt * 512 + lo - j0
                for h in range(NH):
                    ps = sps.tile([128, 512], f32, tag="ss", name="ss")
                    for t in range(2):
                        dc = 2 * h + t
                        nc.tensor.matmul(
                            ps[:, lo:512],
                            qt[:, dc, ic * 128:(ic + 1) * 128],
                            kt[:, dc, jt * 512 + lo:(jt + 1) * 512],
                            start=(t == 0), stop=False)
                    # rank-1 pen: adds -1e9 to !valid columns
                    nc.tensor.matmul(
                        ps[:, lo:512], ones_t[:, :],
                        pen_t[:, jt * 512 + lo:(jt + 1) * 512],
                        start=False, stop=True)
                    # triangular additive mask on the diagonal 128 cols
                    if jt == jt0:
                        nc.vector.tensor_tensor(
                            out=ps[:, lo:lo + 128], in0=ps[:, lo:lo + 128],
                            in1=trir_t.bitcast(f32), op=mybir.AluOpType.add)
                    nc.scalar.activation(
                        p_ts[h][:, dst0:(jt + 1) * 512 - j0],
                        ps[:, lo:512],
                        mybir.ActivationFunctionType.Exp,
                        bias=0.0, scale=1.0,
                        accum_out=sa[:, k, h:h + 1])

            if variant == "nomix":
                nc.sync.dma_start(out=out_d[ic * 128:(ic + 1) * 128, j0:],
                                  in_=p_ts[0][:, :])
                continue

            # w_h = u_h / (gden * (S_h + 1e-30))
            s4 = wrk.tile([128, NH], f32, tag="s4", name="s4")
            if jt0 == 0:
                nc.vector.tensor_tensor(out=s4[:, :], in0=sa[:, 0, :],
                                        in1=sa[:, 1, :], op=mybir.AluOpType.add)
            else:
                nc.vector.tensor_copy(s4[:, :], sa[:, 0, :])
            m4 = wrk.tile([128, NH], f32, tag="m4", name="m4")
            nc.vector.tensor_scalar(
                out=m4[:, :], in0=s4[:, :], scalar1=1e-30,
                scalar2=gd_t[:, ic:ic + 1],
                op0=mybir.AluOpType.add, op1=mybir.AluOpType.mult)
            r4 = wrk.tile([128, NH], f32, tag="r4", name="r4")
            nc.vector.reciprocal(out=r4[:, :], in_=m4[:, :])
            w4 = wrk.tile([128, NH], f32, tag="w4", name="w4")
            nc.vector.tensor_tensor(out=w4[:, :], in0=u_t[:, ic, :],
                                    in1=r4[:, :], op=mybir.AluOpType.mult)

            # mixture: acc = sum_h p_h * w_h
            acc = wrk.tile([128, W], f32, tag="acc", name="acc")
            nc.vector.tensor_scalar(
                out=acc[:, :], in0=p_ts[0][:, :], scalar1=w4[:, 0:1],
                scalar2=None, op0=mybir.AluOpType.mult)
            for h in range(1, NH):
                nc.vector.scalar_tensor_tensor(
                    out=acc[:, :], in0=p_ts[h][:, :], scalar=w4[:, h:h + 1],
                    in1=acc[:, :], op0=mybir.AluOpType.mult,
                    op1=mybir.AluOpType.add)

            # out = ln(acc + 1e-38), then min-masks -> exact -1e9
            o_t = wrk.tile([128, W], f32, tag="o", name="o")
            nc.scalar.activation(o_t[:, :], acc[:, :],
                                 mybir.ActivationFunctionType.Ln,
                                 bias=eps_t[:, 0:1], scale=1.0)
            nc.vector.tensor_tensor(out=o_t[:, :], in0=o_t[:, :],
                                    in1=mvb[:, j0:], op=mybir.AluOpType.min)
            nc.vector.tensor_tensor(out=o_t[:, 0:128], in0=o_t[:, 0:128],
                                    in1=tribig_t[:, :], op=mybir.AluOpType.min)
            nc.scalar.dma_start(out=out_d[ic * 128:(ic + 1) * 128, j0:],
                                in_=o_t[:, :])


_NC_CACHE = None


def _get_nc():
    global _NC_CACHE
    if _NC_CACHE is None:
        _NC_CACHE = build_nc()
    return _NC_CACHE


def make_in_maps(features, valid_mask, Wq, bq, Wk, bk, Wg, bg):
    features = np.asarray(features, dtype=np.float32)
    valid_mask = np.asarray(valid_mask).astype(bool)
    wq_r = round_f32r(np.asarray(Wq, np.float32) / 16.0)
    wk_r = round_f32r(np.asarray(Wk, np.float32))
    wg_r = round_f32r(np.asarray(Wg, np.float32))
    bq_s = (np.asarray(bq, np.float32) / 16.0).reshape(NC, 128).T.copy()
    bk_s = np.asarray(bk, np.float32).reshape(NC, 128).T.copy()
    bg_r = round_f32r(np.asarray(bg, np.float32).reshape(1, NH))
    ones = np.ones((1, 128), np.float32)
    c = np.arange(128)[None, :]
    rr = np.arange(128)[:, None]
    tri = np.where(c > rr, 0.0, NEG).astype(np.float32)
    tribig = np.where(c > rr, BIG, NEG).astype(np.float32)
    cp = np.concatenate([tri, tribig, bq_s, bk_s], axis=1)
    ident = np.eye(128, dtype=np.float32)
    im = np.concatenate([ident, round_f32r(tri)], axis=1)
    in_maps = []
    for b_i in range(B):
        pen = np.where(valid_mask[b_i], 0.0, NEG).astype(np.float32).reshape(1, N)
        mv = np.where(valid_mask[b_i], BIG, NEG).astype(np.float32).reshape(1, N)
        rp = np.concatenate([round_f32r(pen), ones, bg_r], axis=1)
        in_maps.append({
            "ft": round_f32r(features[b_i].T),
            "wq": wq_r, "wk": wk_r, "wg": wg_r,
            "cp": cp, "rp": rp, "mv": mv, "im": im,
        })
    return in_maps


def gather_out(results):
    out = np.empty((B, N, N), dtype=np.float32)
    for b_i in range(B):
        out[b_i] = results[b_i]["out"]
    # block-lower-left region is never written on device
    i_blk = np.arange(N)[:, None] // 128
    dead = np.arange(N)[None, :] < i_blk * 128
    out[:, dead] = np.float32(NEG)
    return out


def kernel(features, valid_mask, Wq, bq, Wk, bk, Wg, bg):
    nc = _get_nc()
    in_maps = make_in_maps(features, valid_mask, Wq, bq, Wk, bk, Wg, bg)
    res = run_bass_kernel_spmd(nc, in_maps, core_ids=list(range(B)))
    return gather_out(res.results)

